# revision 1
# baseline (speedup 1.0000x reference)
"""HardBatchTripletLoss Trainium2 kernel.

Math:
  dist2[i,j] = sq[i] + sq[j] - 2*x_i.x_j
  hardest_pos[i] = max_{j: cls j == cls i} dist(i,j)
  hardest_neg[i] = min_{j: cls j != cls i} dist(i,j)
  loss = mean(relu(hardest_pos - hardest_neg + 1))

Device strategy (8 cores, SPMD, anchor-sharded: core k owns 1024 anchors):
  - Host sorts rows by class. Columns (candidates j) are, per core, the full
    8192 sorted rows *rotated* by 1024k so every core's same-class "band"
    falls in the same chunk indices (63, 0..8) -> one SPMD program.
  - PE: per j-chunk c (128 j's), psum[j, i] = R = 2*x_j.x_i  (transposed
    layout: j on partitions, anchors i on free dim), float32r matmuls.
  - DVE: one fused scalar_tensor_tensor per chunk:
        NR = max(NR, psum - sq_j)        # V = R - sq_j;  d2 = sq_i - V
    Band chunks additionally poison same-class entries with -BIG (via a
    class-equality mask) and min-combine the poisoned window into P:
        P = min(P, psum_poisoned - sq_j)  # same-class entries = V - BIG win
  - ACT offloads part of the PSUM drain: for non-band chunks assigned to it,
    activation(psum, scale=1, bias=-sq_j) -> fp16 tile, min-folded on DVE at
    2x rate. Balances the drain across both engines.
  - Final: PE-transpose NR/P 128x128 slices, DVE reduce over free dim ->
    out[128, 16] per core (cols 0:8 neg-max, 8:16 pos-min).
  Host: hn2 = sq_i - negmax, hp2 = sq_i - (posmin + BIG); sqrt/relu/mean.
"""

import os
import sys
from contextlib import ExitStack

import numpy as np

if "/opt/trn_rl_repo" not in sys.path:
    sys.path.insert(0, "/opt/trn_rl_repo")

N = 8192
D = 128
NCORES = 8
A = N // NCORES  # anchors per core = 1024
NCHUNK = N // 128  # 64 j-chunks
BIG = 16384.0
# Band: chunks whose columns can share a class with this core's anchors.
# Window of anchors possibly same-class with chunk c's columns (maxcnt<=64).
BAND = [(63, 0, 64)] + [
    (c, max(0, 128 * c - 64), min(A, 128 * c + 192)) for c in range(9)
]
BAND_SET = {c for c, _, _ in BAND}

# feature dtype for the matmul: "f32r" (full-rate fp32), "bf16", "f32"
FEAT = os.environ.get("TRIP_FEAT", "f32r")
# how many non-band chunks ACT drains (fp16 path); rest go via DVE STT
NACT = int(os.environ.get("TRIP_NACT", "36"))

_CACHE = {}


def _emit_body(nc, tc, pools, aps, mybir):
    dt = mybir.dt
    Alu = mybir.AluOpType
    feat_dt = {"f32r": dt.float32r, "bf16": dt.bfloat16, "f32": dt.float32}[FEAT]
    const, psum, tpsum, eqp, actp = pools
    xall, anch, sqc, clsc, clsa, ident, out = aps
    band_map = {c: (w0, w1) for c, w0, w1 in BAND}

    xall_sb = const.tile([128, N], feat_dt, tag="xall")
    anch_sb = const.tile([128, A], feat_dt, tag="anch")
    sqc_sb = const.tile([128, NCHUNK], dt.float32, tag="sqc")
    nsqc_sb = const.tile([128, NCHUNK], dt.float32, tag="nsqc")
    clsc_sb = const.tile([128, NCHUNK], dt.float32, tag="clsc")
    clsa_sb = const.tile([128, A], dt.float32, tag="clsa")
    ident_sb = const.tile([128, 128], dt.float32, tag="ident")
    nc.sync.dma_start(anch_sb[:], anch[:])
    nc.sync.dma_start(sqc_sb[:], sqc[:])
    nc.sync.dma_start(clsc_sb[:], clsc[:])
    nc.sync.dma_start(clsa_sb[:], clsa[:])
    nc.sync.dma_start(ident_sb[:], ident[:])
    # negated sq for the ACT bias path
    nc.gpsimd.tensor_scalar(
        out=nsqc_sb[:], in0=sqc_sb[:], scalar1=-1.0, scalar2=None, op0=Alu.mult
    )
    # load xall in slices so early matmuls can start sooner
    for s in range(8):
        nc.sync.dma_start(
            xall_sb[:, s * 1024 : (s + 1) * 1024],
            xall[:, s * 1024 : (s + 1) * 1024],
        )

    NR = const.tile([128, A], dt.float32, tag="NR")
    P = const.tile([128, A], dt.float32, tag="P")
    NR16 = const.tile([128, A], dt.float16, tag="NR16")
    outsb = const.tile([128, 16], dt.float32, tag="outsb")
    nc.gpsimd.memset(NR[:], -1e38)
    nc.gpsimd.memset(P[:], 1e38)
    if NACT:
        nc.gpsimd.memset(NR16[:], -60000.0)

    # which non-band chunks use the ACT drain path (spread them out)
    nonband = [c for c in range(NCHUNK) if c not in BAND_SET]
    if NACT:
        idx = np.linspace(0, len(nonband) - 1, NACT).round().astype(int)
        act_chunks = {nonband[i] for i in idx}
    else:
        act_chunks = set()

    for c in range(NCHUNK):
        ps = psum.tile([128, A], dt.float32, tag="ps")
        for h in range(A // 512):
            nc.tensor.matmul(
                ps[:, h * 512 : (h + 1) * 512],
                lhsT=xall_sb[:, c * 128 : (c + 1) * 128],
                rhs=anch_sb[:, h * 512 : (h + 1) * 512],
                start=True,
                stop=True,
            )
        if c in band_map:
            w0, w1 = band_map[c]
            eq = eqp.tile([128, 256], dt.float32, tag="eq")
            weq = eq[:, : w1 - w0]
            nc.vector.tensor_scalar(
                out=weq,
                in0=clsa_sb[:, w0:w1],
                scalar1=clsc_sb[:, c : c + 1],
                scalar2=None,
                op0=Alu.is_equal,
            )
            # poison same-class entries: ps_w += -BIG * eq
            nc.vector.scalar_tensor_tensor(
                out=ps[:, w0:w1],
                in0=weq,
                scalar=-BIG,
                in1=ps[:, w0:w1],
                op0=Alu.mult,
                op1=Alu.add,
            )
            # pos: P_w = min(P_w, ps_w - sq_j)
            nc.vector.scalar_tensor_tensor(
                out=P[:, w0:w1],
                in0=ps[:, w0:w1],
                scalar=sqc_sb[:, c : c + 1],
                in1=P[:, w0:w1],
                op0=Alu.subtract,
                op1=Alu.min,
            )
        if c in act_chunks:
            # ACT drain: fp16 tile = psum - sq_j, folded into NR16 on DVE @2x
            at = actp.tile([128, A], dt.float16, tag="at")
            nc.scalar.activation(
                at[:],
                ps[:],
                mybir.ActivationFunctionType.Identity,
                bias=nsqc_sb[:, c : c + 1],
                scale=1.0,
            )
            nc.vector.tensor_tensor(
                out=NR16[:], in0=at[:], in1=NR16[:], op=Alu.max
            )
        else:
            # neg: NR = max(NR, ps - sq_j)
            nc.vector.scalar_tensor_tensor(
                out=NR[:],
                in0=ps[:],
                scalar=sqc_sb[:, c : c + 1],
                in1=NR[:],
                op0=Alu.subtract,
                op1=Alu.max,
            )

    if NACT:
        # fold the fp16 running max into NR (fp32)
        nc.vector.tensor_tensor(out=NR[:], in0=NR16[:], in1=NR[:], op=Alu.max)

    for t in range(8):
        tp = tpsum.tile([128, 128], dt.float32, tag="tp")
        nc.tensor.transpose(tp[:], NR[:, t * 128 : (t + 1) * 128], ident_sb[:])
        nc.vector.tensor_reduce(
            out=outsb[:, t : t + 1],
            in_=tp[:],
            axis=mybir.AxisListType.X,
            op=Alu.max,
        )
        tp2 = tpsum.tile([128, 128], dt.float32, tag="tp")
        nc.tensor.transpose(tp2[:], P[:, t * 128 : (t + 1) * 128], ident_sb[:])
        nc.vector.tensor_reduce(
            out=outsb[:, 8 + t : 9 + t],
            in_=tp2[:],
            axis=mybir.AxisListType.X,
            op=Alu.min,
        )
    nc.sync.dma_start(out[:], outsb[:])


def _build_program(rep=1):
    import concourse.mybir as mybir
    import concourse.tile as tile
    from concourse import bacc

    dt = mybir.dt
    feat_dt = {"f32r": dt.float32r, "bf16": dt.bfloat16, "f32": dt.float32}[FEAT]

    nc = bacc.Bacc(
        "TRN2", target_bir_lowering=False, debug=False, num_devices=NCORES
    )

    xall = nc.dram_tensor("xall", [128, N], feat_dt, kind="ExternalInput")
    anch = nc.dram_tensor("anch", [128, A], feat_dt, kind="ExternalInput")
    sqc = nc.dram_tensor("sqc", [128, NCHUNK], dt.float32, kind="ExternalInput")
    clsc = nc.dram_tensor("clsc", [128, NCHUNK], dt.float32, kind="ExternalInput")
    clsa = nc.dram_tensor("clsa", [128, A], dt.float32, kind="ExternalInput")
    ident = nc.dram_tensor("ident", [128, 128], dt.float32, kind="ExternalInput")
    out = nc.dram_tensor("out", [128, 16], dt.float32, kind="ExternalOutput")
    aps = (xall, anch, sqc, clsc, clsa, ident, out)

    with ExitStack() as ctx:
        tc = ctx.enter_context(tile.TileContext(nc))
        cbufs = 1 if rep == 1 else 2
        const = ctx.enter_context(tc.tile_pool(name="const", bufs=cbufs))
        psum = ctx.enter_context(tc.tile_pool(name="psum", bufs=3, space="PSUM"))
        tpsum = ctx.enter_context(tc.tile_pool(name="tpsum", bufs=2, space="PSUM"))
        eqp = ctx.enter_context(tc.tile_pool(name="eqp", bufs=2))
        actp = ctx.enter_context(tc.tile_pool(name="actp", bufs=4))
        pools = (const, psum, tpsum, eqp, actp)
        for _ in range(rep):
            _emit_body(nc, tc, pools, aps, mybir)

    nc.finalize()
    return nc


class _Runner:
    """Mirror of bass2jax.run_bass_via_pjrt's multi-core branch, built once
    so repeated executions reuse the same jitted callable and device-resident
    inputs (timing then measures NEFF execution + dispatch only)."""

    def __init__(self, nc):
        import jax
        import concourse.mybir as mybir
        from concourse import bass2jax
        from jax.sharding import Mesh, NamedSharding, PartitionSpec
        from jax.experimental.shard_map import shard_map

        self.jax = jax
        bass2jax.install_neuronx_cc_hook()
        partition_name = (
            nc.partition_id_tensor.name if nc.partition_id_tensor else None
        )
        in_names, out_names, out_avals, zero_outs = [], [], [], []
        for alloc in nc.m.functions[0].allocations:
            if not isinstance(alloc, mybir.MemoryLocationSet):
                continue
            name = alloc.memorylocations[0].name
            if alloc.kind == "ExternalInput":
                if name != partition_name:
                    in_names.append(name)
            elif alloc.kind == "ExternalOutput":
                out_names.append(name)
                shape = tuple(alloc.tensor_shape)
                dtype = mybir.dt.np(alloc.dtype)
                out_avals.append(jax.core.ShapedArray(shape, dtype))
                zero_outs.append(np.zeros(shape, dtype))
        n_params = len(in_names)
        n_outs = len(out_avals)
        all_in_names = list(in_names) + list(out_names)
        if partition_name is not None:
            all_in_names.append(partition_name)

        def _body(*args):
            operands = list(args)
            if partition_name is not None:
                operands.append(bass2jax.partition_id_tensor())
            outs = bass2jax._bass_exec_p.bind(
                *operands,
                out_avals=tuple(out_avals),
                in_names=tuple(all_in_names),
                out_names=tuple(out_names),
                lowering_input_output_aliases=(),
                sim_require_finite=True,
                sim_require_nnan=True,
                nc=nc,
            )
            return tuple(outs)

        devices = jax.devices()[:NCORES]
        mesh = Mesh(np.asarray(devices), ("core",))
        in_specs = (PartitionSpec("core"),) * (n_params + n_outs)
        out_specs = (PartitionSpec("core"),) * n_outs
        donate = tuple(range(n_params, n_params + n_outs))
        self.fn = jax.jit(
            shard_map(
                _body,
                mesh=mesh,
                in_specs=in_specs,
                out_specs=out_specs,
                check_rep=False,
            ),
            donate_argnums=donate,
            keep_unused=True,
        )
        self.mesh = mesh
        self.sharding = NamedSharding(mesh, PartitionSpec("core"))
        self.in_names = in_names
        self.out_names = out_names
        self.out_avals = out_avals
        self.zero_outs = zero_outs
        self.n_params = n_params

    def put_inputs(self, in_maps):
        concat_in = [
            np.concatenate([np.asarray(m[name]) for m in in_maps], axis=0)
            for name in self.in_names
        ]
        return [self.jax.device_put(x, self.sharding) for x in concat_in]

    def exec_once(self, dev_in):
        zeros = [
            np.zeros((NCORES * z.shape[0], *z.shape[1:]), z.dtype)
            for z in self.zero_outs
        ]
        out = self.fn(*dev_in, *zeros)
        self.jax.block_until_ready(out)
        return out

    def run(self, in_maps):
        out_arrs = self.exec_once(self.put_inputs(in_maps))
        return [
            {
                name: np.asarray(out_arrs[i]).reshape(
                    NCORES, *self.out_avals[i].shape
                )[c]
                for i, name in enumerate(self.out_names)
            }
            for c in range(NCORES)
        ]


def _get_runner():
    if "runner" not in _CACHE:
        _CACHE["runner"] = _Runner(_build_program())
    return _CACHE["runner"]


def _np_feat(x):
    if FEAT == "bf16":
        import ml_dtypes

        return np.ascontiguousarray(x, dtype=ml_dtypes.bfloat16)
    return np.ascontiguousarray(x, dtype=np.float32)


def _prep_in_maps(feats, tgts):
    order = np.argsort(tgts, kind="stable")
    xs = np.ascontiguousarray(feats[order])
    ts_ = np.asarray(tgts)[order].astype(np.int64)
    assert np.bincount(ts_).max() <= 64, "class-size bound for band width"
    sq = (xs.astype(np.float64) ** 2).sum(1).astype(np.float32)
    cls_f = ts_.astype(np.float32)
    ident = np.eye(128, dtype=np.float32)
    in_maps = []
    for k in range(NCORES):
        rot = (np.arange(N) + A * k) % N
        in_maps.append(
            {
                "xall": _np_feat(xs[rot].T),
                "anch": _np_feat(2.0 * xs[A * k : A * (k + 1)].T),
                "sqc": np.ascontiguousarray(sq[rot].reshape(NCHUNK, 128).T),
                "clsc": np.ascontiguousarray(cls_f[rot].reshape(NCHUNK, 128).T),
                "clsa": np.ascontiguousarray(
                    np.broadcast_to(cls_f[A * k : A * (k + 1)], (128, A))
                ),
                "ident": ident,
            }
        )
    return in_maps, sq


def _finish(results, sq):
    hp_sq = np.empty(N, np.float64)
    hn_sq = np.empty(N, np.float64)
    for k in range(NCORES):
        o = np.asarray(results[k]["out"], dtype=np.float64)
        negmax = o[:, :8].T.reshape(A)
        posmin = o[:, 8:].T.reshape(A)
        sqa = sq[A * k : A * (k + 1)].astype(np.float64)
        hn_sq[A * k : A * (k + 1)] = sqa - negmax
        hp_sq[A * k : A * (k + 1)] = sqa - (posmin + BIG)
    hp = np.sqrt(np.maximum(hp_sq, 0.0))
    hn = np.sqrt(np.maximum(hn_sq, 0.0))
    return np.float32(np.maximum(hp - hn + 1.0, 0.0).mean())


def kernel(features, targets):
    feats = np.asarray(features, dtype=np.float32)
    tgts = np.asarray(targets)
    assert feats.shape == (N, D)
    in_maps, sq = _prep_in_maps(feats, tgts)
    results = _get_runner().run(in_maps)
    return _finish(results, sq)


def time_exec(features, targets, iters=10, rep=9):
    """Per-iteration kernel time via (wall(rep) - wall(1)) / (rep - 1); the
    ~88ms axon RPC overhead cancels in the subtraction."""
    import time

    feats = np.asarray(features, dtype=np.float32)
    in_maps, _ = _prep_in_maps(feats, np.asarray(targets))

    def bench(runner):
        dev_in = runner.put_inputs(in_maps)
        runner.exec_once(dev_in)  # warmup
        ts = []
        for _ in range(iters):
            t0 = time.perf_counter()
            runner.exec_once(dev_in)
            ts.append((time.perf_counter() - t0) * 1e9)
        return ts

    r1 = _get_runner()
    if "runner_rep" not in _CACHE:
        _CACHE["runner_rep"] = _Runner(_build_program(rep=rep))
    ts1 = bench(r1)
    tsR = bench(_CACHE["runner_rep"])
    per_iter = (min(tsR) - min(ts1)) / (rep - 1)
    return per_iter, ts1, tsR

